# revision 1
# baseline (speedup 1.0000x reference)
"""DiffAttention Trainium2 kernel.

Math (per batch b, head h):
  q,k split into two streams of D=64; v has E=128 channels.
  attn_s = softmax_causal(q_s k_s^T / 8) @ v            (s = 1,2)
  lam    = exp(lq1.lk1) - exp(lq2.lk2) + 0.8            (host scalar)
  x      = attn1 - lam*attn2
  out    = 0.2 * w * x * rsqrt(mean_e(x^2) + eps)

Kernel strategy (8 cores, SPMD):
  - 16 independent (b,h) attention problems; 2 per core.
  - Host pre-packs per core: Q^T/K^T as [128, L] (streams stacked on the
    partition axis: rows 0..63 = stream1 dims, 64..127 = stream2), and
    V with a leading ones-column [L, 129] so each PV matmul also produces
    the softmax denominator in column 0.
  - On device we compute S^T tiles [k_part, q_free] = K_tile^T.T @ Q^T so
    the exp'd probabilities P^T are directly the lhsT of the PV matmul,
    giving the output in natural [q, e] layout with zero transposes.
  - Softmax is computed without max-subtraction (scores ~ N(0,1) after
    scaling, bounded by ~±7; exp stays in fp32 range), and normalization
    is deferred past the stream combine using RMSNorm scale-invariance:
      x ∝ U1*den2 - lam*U2*den1   (U_s unnormalized, den_s row sums)
  - rsqrt for the RMSNorm is exp(-0.5*ln(ms)) so every ACT op uses one
    activation table set (natural_log_exp_and_others) -- no table thrash.
  - Causal masking: only lower-triangle k-tiles are computed; diagonal
    tiles are exp'd then zeroed above the diagonal with a GPSIMD
    affine_select (post-exp zeroing is exact).
"""

from contextlib import ExitStack

import numpy as np

import concourse.bass as bass
import concourse.mybir as mybir
from concourse import bacc
from concourse._compat import axon_active
from concourse.bass import MemorySpace
from concourse.bass_utils import run_bass_kernel_spmd
from concourse.tile import TileContext

F32 = mybir.dt.float32
BF16 = mybir.dt.bfloat16
AF = mybir.ActivationFunctionType
ALU = mybir.AluOpType

B, L, H, D = 2, 2048, 8, 64
E = 2 * D               # 128 v-channels per head
NP = 128                # SBUF partitions
PAIRS = 2               # (b,h) pairs per core
N_CORES = 8
QB = 512                # q columns per block (4 tiles of 128)
NQB = L // QB           # 4
CHUNK = 2               # k-tiles exp'd per ACT op (exp width = CHUNK*QB)
NKT = L // NP           # 16 k tiles
LAMBDA_INIT = 0.8
EPS = 1e-5
OUT_SCALE = 1.0 - LAMBDA_INIT  # 0.2
SM_SCALE = 1.0 / 8.0


def _build_program(w_is_ones: bool, repeat: int = 1, skip: frozenset = frozenset()) -> bass.Bass:
    nc = bacc.Bacc(
        "TRN2",
        target_bir_lowering=False,
        debug=not axon_active(),
        enable_asserts=False,
        num_devices=N_CORES,
    )
    qt_d = nc.declare_dram_parameter("qt", [PAIRS, NP, L], BF16, isOutput=False)
    kt_d = nc.declare_dram_parameter("kt", [PAIRS, NP, L], BF16, isOutput=False)
    vv_d = nc.declare_dram_parameter("vv", [PAIRS, L, E + 1], BF16, isOutput=False)
    lam_d = nc.declare_dram_parameter("lam", [NP, 1], F32, isOutput=False)
    if not w_is_ones:
        wb_d = nc.declare_dram_parameter("wb", [NP, E], F32, isOutput=False)
    out_d = nc.declare_dram_parameter("out", [PAIRS, L, E], F32, isOutput=True)

    with TileContext(nc) as tc, ExitStack() as ctx:
        const = ctx.enter_context(tc.tile_pool(name="const", bufs=1))
        io = ctx.enter_context(tc.tile_pool(name="io", bufs=2))
        ptp = ctx.enter_context(tc.tile_pool(name="ptp", bufs=3))
        ep = ctx.enter_context(tc.tile_pool(name="ep", bufs=2))
        xp = ctx.enter_context(tc.tile_pool(name="xp", bufs=2))
        stp = ctx.enter_context(
            tc.tile_pool(name="stp", bufs=1, space=MemorySpace.PSUM)
        )
        up = ctx.enter_context(tc.tile_pool(name="up", bufs=1, space=MemorySpace.PSUM))

        lam_sb = const.tile([NP, 1], F32)
        nc.sync.dma_start(lam_sb[:], lam_d[:])
        magic = const.tile([NP, 1], mybir.dt.int32)
        nc.gpsimd.memset(magic[:], 0x5F3759DF)
        if not w_is_ones:
            wb_sb = const.tile([NP, E], F32)
            nc.sync.dma_start(wb_sb[:], wb_d[:])

        st_par = [0]
        rep_ctx = tc.For_i(0, repeat, 1) if repeat > 1 else None
        if rep_ctx is not None:
            ctx.enter_context(rep_ctx)
        for p in range(PAIRS):
            qt_sb = io.tile([NP, L], BF16, tag="qt")
            kt_sb = io.tile([NP, L], BF16, tag="kt")
            vv_sb = io.tile([NP, NKT, E + 1], BF16, tag="vv")
            nc.sync.dma_start(qt_sb[:], qt_d[p])
            nc.sync.dma_start(kt_sb[:], kt_d[p])
            nc.sync.dma_start(vv_sb[:], vv_d[p].rearrange("(t k) e -> k t e", k=NP))

            x_all = xp.tile([NP, NKT, E], F32, tag="x")       # combined diff
            ms_all = xp.tile([NP, NKT], F32, tag="ms")        # mean squares
            o_sb = xp.tile([NP, NKT, E], F32, tag="o")        # final output

            for qb in range(NQB):
                nki = 4 * qb + 4  # causal: k tiles 0..4qb+3 for q tiles 4qb..4qb+3
                # PSUM ground rules (HW-probed): a K=64 matmul output owns a
                # full bank; concurrently-open accumulation groups must be in
                # different banks. So: each N=512 S^T matmul gets one bank,
                # the 4 PV accumulators get one bank each, and the two
                # streams run sequentially (stream 0's U copied to SBUF).
                u0_sb = ep.tile([NP, 4, E + 1], F32, tag="u0sb")
                for s in range(2):
                    sp = slice(s * D, (s + 1) * D)
                    u_ps = up.tile([NP, 4, 512], F32, tag="u", name="u")
                    for kc in range(0, nki, CHUNK):
                        cn = min(CHUNK, nki - kc)
                        # two ST tag groups double-buffer consecutive chunks
                        # (4 PSUM banks total; not tied to streams)
                        st_par[0] = (st_par[0] + 1) % 2
                        st = stp.tile(
                            [NP, CHUNK, QB], F32,
                            tag=f"st{st_par[0]}", name=f"st{st_par[0]}",
                        )
                        if "s" not in skip:
                            for j in range(cn):
                                ki = kc + j
                                nc.tensor.matmul(
                                    st[:, j, :],
                                    kt_sb[sp, ki * NP : (ki + 1) * NP],
                                    qt_sb[sp, qb * QB : (qb + 1) * QB],
                                    start=True,
                                    stop=True,
                                )
                        pt = ptp.tile([NP, CHUNK, QB], BF16, tag="pt")
                        if "exp" not in skip:
                            nc.scalar.activation(
                                pt[:, :cn, :], st[:, :cn, :], AF.Exp, scale=SM_SCALE
                            )
                        else:  # timing probe: 1-element ACT keeps dataflow
                            nc.scalar.activation(
                                pt[:, 0, 0:1], st[:, 0, 0:1], AF.Exp, scale=SM_SCALE
                            )
                        for j in range(cn):
                            ki = kc + j
                            for qi in range(4):
                                qt_g = 4 * qb + qi
                                if qt_g < ki:
                                    continue
                                if qt_g == ki and "mask" not in skip:
                                    sl = pt[:, j, qi * NP : (qi + 1) * NP]
                                    # keep where q_local >= k_local, else 0
                                    nc.gpsimd.affine_select(
                                        sl,
                                        sl,
                                        pattern=[[1, NP]],
                                        compare_op=ALU.is_ge,
                                        fill=0.0,
                                        base=0,
                                        channel_multiplier=-1,
                                    )
                                if "pv" not in skip:
                                    nc.tensor.matmul(
                                        u_ps[:, qi, 0 : E + 1],
                                        pt[:, j, qi * NP : (qi + 1) * NP],
                                        vv_sb[:, ki, :],
                                        start=(ki == 0),
                                        stop=(qt_g == ki),
                                    )
                    if s == 0 and "pv" not in skip:
                        # free the PSUM banks for stream 1
                        nc.vector.tensor_copy(u0_sb[:], u_ps[:, :, 0 : E + 1])

                # --- combine streams for this q block ---------------------
                # den1 = u0_sb[:, :, 0]; U1 = u0_sb[:, :, 1:]
                # den2 = u_ps[:, :, 0];  U2 = u_ps[:, :, 1:]   (still in PSUM)
                if "epi" in skip:
                    continue
                d2_sb = ep.tile([NP, 4], F32, tag="d2")
                nc.vector.tensor_copy(d2_sb[:], u_ps[:, :, 0])
                d1l = ep.tile([NP, 4], F32, tag="d1l")  # lam * den1
                nc.vector.tensor_scalar(
                    d1l[:], u0_sb[:, :, 0], lam_sb[:, 0:1], None, ALU.mult
                )
                # eps in the reference applies to the *normalized* x, so the
                # deferred-normalization ms needs eps*(den1*den2)^2:
                dd = ep.tile([NP, 4], F32, tag="dd")  # sqrt(eps)*den1*den2
                nc.vector.scalar_tensor_tensor(
                    dd[:],
                    u0_sb[:, :, 0],
                    float(np.sqrt(EPS)),
                    d2_sb[:],
                    ALU.mult,
                    ALU.mult,
                )
                edd = ep.tile([NP, 4], F32, tag="edd")  # eps*(den1*den2)^2
                nc.vector.tensor_tensor(edd[:], dd[:], dd[:], ALU.mult)
                t2 = ep.tile([NP, 4, E], F32, tag="t2")  # lam*den1*U2
                nc.vector.tensor_tensor(
                    t2[:],
                    u_ps[:, :, 1 : E + 1],
                    d1l[:].unsqueeze(2).broadcast_to([NP, 4, E]),
                    ALU.mult,
                )
                for qi in range(4):
                    qt_g = 4 * qb + qi
                    # x = U1*den2 - t2
                    nc.vector.scalar_tensor_tensor(
                        x_all[:, qt_g, :],
                        u0_sb[:, qi, 1 : E + 1],
                        d2_sb[:, qi : qi + 1],
                        t2[:, qi, :],
                        ALU.mult,
                        ALU.subtract,
                    )
                    # ms = mean(x^2) (custom-DVE ttr is broken on HW; use
                    # standard square + reduce, 1/E folded into the square)
                    xsq = ep.tile([NP, E], F32, tag="xsq")
                    nc.vector.scalar_tensor_tensor(
                        xsq[:],
                        x_all[:, qt_g, :],
                        1.0 / E,
                        x_all[:, qt_g, :],
                        ALU.mult,
                        ALU.mult,
                    )
                    nc.vector.reduce_sum(
                        ms_all[:, qt_g : qt_g + 1],
                        xsq[:],
                        axis=mybir.AxisListType.X,
                    )
                # ms += eps*(den1*den2)^2
                nc.vector.tensor_tensor(
                    ms_all[:, 4 * qb : 4 * qb + 4],
                    ms_all[:, 4 * qb : 4 * qb + 4],
                    edd[:],
                    ALU.add,
                )

            # --- per-pair finale: rs = 0.2 * rsqrt(ms), out = x * rs * w --
            if "epi" in skip:
                nc.gpsimd.memset(o_sb[:], 0.0)
                nc.sync.dma_start(
                    out_d[p].rearrange("(t q) e -> q t e", q=NP), o_sb[:]
                )
                continue
            # rs = 0.2*rsqrt(ms) entirely on DVE (quake seed + 3 Newton
            # iterations; quadratic convergence -> fp32-accurate). Keeping
            # the scalar engine exp-only avoids ACT table-set thrash.
            I32 = mybir.dt.int32
            sh = ep.tile([NP, NKT], I32, tag="sh")
            nc.vector.tensor_scalar(
                sh[:], ms_all[:].bitcast(I32), 1, None, ALU.logical_shift_right
            )
            y = ep.tile([NP, NKT], F32, tag="y")
            nc.vector.tensor_tensor(
                y[:].bitcast(I32),
                magic[:].broadcast_to([NP, NKT]).bitcast(I32),
                sh[:],
                ALU.subtract,
            )
            mh = ep.tile([NP, NKT], F32, tag="mh")  # 0.5*ms
            nc.vector.tensor_scalar(mh[:], ms_all[:], 0.5, None, ALU.mult)
            for it in range(3):
                yy = ep.tile([NP, NKT], F32, tag="yy", name="yy")
                nc.vector.tensor_tensor(yy[:], y[:], y[:], ALU.mult)
                nc.vector.tensor_tensor(yy[:], yy[:], mh[:], ALU.mult)
                # c = (yy - 1.5) * (-scale): last iteration folds the 0.2
                fin = -OUT_SCALE if it == 2 else -1.0
                nc.vector.tensor_scalar(
                    yy[:], yy[:], 1.5, fin, ALU.subtract, ALU.mult
                )
                yn = ep.tile([NP, NKT], F32, tag="yn", name="yn")
                nc.vector.tensor_tensor(yn[:], y[:], yy[:], ALU.mult)
                y = yn
            rs = y
            nc.vector.tensor_tensor(
                o_sb[:],
                x_all[:],
                rs[:].unsqueeze(2).broadcast_to([NP, NKT, E]),
                ALU.mult,
            )
            if not w_is_ones:
                nc.vector.tensor_tensor(
                    o_sb[:],
                    o_sb[:],
                    wb_sb[:].unsqueeze(1).broadcast_to([NP, NKT, E]),
                    ALU.mult,
                )
            nc.sync.dma_start(
                out_d[p].rearrange("(t q) e -> q t e", q=NP), o_sb[:]
            )

    nc.compile()
    return nc


_PROGRAM_CACHE: dict = {}


def _get_program(w_is_ones: bool, repeat: int = 1) -> bass.Bass:
    key = (w_is_ones, repeat)
    if key not in _PROGRAM_CACHE:
        _PROGRAM_CACHE[key] = _build_program(w_is_ones, repeat)
    return _PROGRAM_CACHE[key]


def make_in_maps(query, key, value, lambda_q1, lambda_k1, lambda_q2, lambda_k2,
                 sub_norm_w):
    """Host-side shard/pack. Returns (in_maps, w_is_ones)."""
    query = np.asarray(query, dtype=np.float32)
    key = np.asarray(key, dtype=np.float32)
    value = np.asarray(value, dtype=np.float32)
    lam = float(
        np.exp(np.sum(np.float64(lambda_q1) * np.float64(lambda_k1)))
        - np.exp(np.sum(np.float64(lambda_q2) * np.float64(lambda_k2)))
        + LAMBDA_INIT
    )
    w = np.asarray(sub_norm_w, dtype=np.float32)
    w_is_ones = bool(np.all(w == 1.0))

    import ml_dtypes

    bf16 = ml_dtypes.bfloat16
    q5 = query.reshape(B, L, H, 2 * D)
    k5 = key.reshape(B, L, H, 2 * D)
    v4 = value.reshape(B, L, H, E)
    lam_arr = np.full((NP, 1), lam, dtype=np.float32)
    wb = np.broadcast_to(w[None, :], (NP, E)).copy() if not w_is_ones else None

    in_maps = []
    for c in range(N_CORES):
        qt = np.empty((PAIRS, NP, L), dtype=bf16)
        kt = np.empty((PAIRS, NP, L), dtype=bf16)
        vv = np.empty((PAIRS, L, E + 1), dtype=bf16)
        for p in range(PAIRS):
            f = c * PAIRS + p
            b, h = divmod(f, H)
            qt[p] = q5[b, :, h].T.astype(bf16)
            kt[p] = k5[b, :, h].T.astype(bf16)
            vv[p, :, 0] = 1.0
            vv[p, :, 1:] = v4[b, :, h].astype(bf16)
        m = {"qt": qt, "kt": kt, "vv": vv, "lam": lam_arr}
        if not w_is_ones:
            m["wb"] = wb
        in_maps.append(m)
    return in_maps, w_is_ones


def assemble_output(results) -> np.ndarray:
    out = np.empty((B, L, H * E), dtype=np.float32)
    for c in range(N_CORES):
        o = results[c]["out"]
        for p in range(PAIRS):
            f = c * PAIRS + p
            b, h = divmod(f, H)
            out[b, :, h * E : (h + 1) * E] = o[p]
    return out


def kernel(query, key, value, lambda_q1, lambda_k1, lambda_q2, lambda_k2,
           sub_norm_w, **_unused):
    in_maps, w_is_ones = make_in_maps(
        query, key, value, lambda_q1, lambda_k1, lambda_q2, lambda_k2, sub_norm_w
    )
    nc = _get_program(w_is_ones)
    res = run_bass_kernel_spmd(nc, in_maps, core_ids=list(range(N_CORES)))
    return assemble_output(res.results)



# revision 2
# speedup vs baseline: 1.1534x; 1.1534x over previous
"""DiffAttention Trainium2 kernel.

Math (per batch b, head h):
  q,k split into two streams of D=64; v has E=128 channels.
  attn_s = softmax_causal(q_s k_s^T / 8) @ v            (s = 1,2)
  lam    = exp(lq1.lk1) - exp(lq2.lk2) + 0.8            (host scalar)
  x      = attn1 - lam*attn2
  out    = 0.2 * w * x * rsqrt(mean_e(x^2) + eps)

Structure (measured ~110us on 8 axon trn2 cores, vs 151us baseline):
  - Stream-paired S matmuls: the two q/k streams live on SBUF partitions
    0:64 and 64:128, so their K=64 S-tile matmuls get tile_position
    (0,0) and (64,0) automatically. Issued adjacently, the PE runs them
    CONCURRENTLY in the two row-halves of the array (measured ~2x: a
    K=64 matmul alone runs at half rate, 0.95 ns/row vs 0.5 for K=128).
  - Causal-trimmed S matmuls and exps: diag-regime k-tiles (ki>4qb) only
    compute/exp q-cols >= (ki-4qb)*128. Saves ~15% PE rows + ACT elems.
  - Per qb: all S+exp first (per-ki both-stream chunks through 2
    double-buffered 2-bank PSUM st tiles), then PV per stream with
    sequential per-qi accumulation groups (K=128, full rate).
  - Epilogue fuses mean-square into the xsq pass via stt accum_out
    (one pass less + no reduce instrs).
  - Optional: every Nth off-diag exp chunk computed on DVE via a
    bf16-Schraudolph bit-trick (int16 = x*A+B, bitcast bf16), offloading
    the ACT exp bottleneck. dve_every=0 disables.
"""

from contextlib import ExitStack

import numpy as np

import concourse.bass as bass
import concourse.mybir as mybir
from concourse import bacc
from concourse._compat import axon_active
from concourse.bass import MemorySpace
from concourse.bass_utils import run_bass_kernel_spmd
from concourse.tile import TileContext

F32 = mybir.dt.float32
BF16 = mybir.dt.bfloat16
I16 = mybir.dt.int16
I32 = mybir.dt.int32
AF = mybir.ActivationFunctionType
ALU = mybir.AluOpType

B, L, H, D = 2, 2048, 8, 64
E = 2 * D               # 128 v-channels per head
NP = 128                # SBUF partitions
PAIRS = 2               # (b,h) pairs per core
N_CORES = 8
QB = 512                # q columns per block (4 tiles of 128)
NQB = L // QB           # 4
CHUNK = 2               # k-tiles exp'd per ACT op
NKT = L // NP           # 16 k tiles
LAMBDA_INIT = 0.8
EPS = 1e-5
OUT_SCALE = 1.0 - LAMBDA_INIT  # 0.2
SM_SCALE = 1.0 / 8.0

# bf16 Schraudolph: i16 = round(x*SM_SCALE*log2(e)*128 + (127-sigma)*128),
# bitcast as bf16 gives ~exp(x/8) with ~3% max rel error.
SCHRAU_A = float(128.0 * SM_SCALE * np.log2(np.e))
SCHRAU_B = float(128.0 * (127.0 - 0.0436775))


def _build_program(w_is_ones: bool, repeat: int = 1, skip: frozenset = frozenset(),
                   dve_every: int = 0) -> bass.Bass:
    nc = bacc.Bacc(
        "TRN2",
        target_bir_lowering=False,
        debug=not axon_active(),
        enable_asserts=False,
        num_devices=N_CORES,
    )
    qt_d = nc.declare_dram_parameter("qt", [PAIRS, NP, L], BF16, isOutput=False)
    kt_d = nc.declare_dram_parameter("kt", [PAIRS, NP, L], BF16, isOutput=False)
    vv_d = nc.declare_dram_parameter("vv", [PAIRS, L, E + 1], BF16, isOutput=False)
    lam_d = nc.declare_dram_parameter("lam", [NP, 1], F32, isOutput=False)
    if not w_is_ones:
        wb_d = nc.declare_dram_parameter("wb", [NP, E], F32, isOutput=False)
    out_d = nc.declare_dram_parameter("out", [PAIRS, L, E], F32, isOutput=True)

    with TileContext(nc) as tc, ExitStack() as ctx:
        const = ctx.enter_context(tc.tile_pool(name="const", bufs=1))
        io = ctx.enter_context(tc.tile_pool(name="io", bufs=2))
        ptp = ctx.enter_context(tc.tile_pool(name="ptp", bufs=3))
        ep = ctx.enter_context(tc.tile_pool(name="ep", bufs=2))
        xp = ctx.enter_context(tc.tile_pool(name="xp", bufs=2))
        stp = ctx.enter_context(
            tc.tile_pool(name="stp", bufs=1, space=MemorySpace.PSUM)
        )
        up = ctx.enter_context(tc.tile_pool(name="up", bufs=1, space=MemorySpace.PSUM))

        lam_sb = const.tile([NP, 1], F32)
        nc.sync.dma_start(lam_sb[:], lam_d[:])
        magic = const.tile([NP, 1], mybir.dt.int32)
        nc.gpsimd.memset(magic[:], 0x5F3759DF)
        if not w_is_ones:
            wb_sb = const.tile([NP, E], F32)
            nc.sync.dma_start(wb_sb[:], wb_d[:])

        st_par = [0]     # rotates the 2 st PSUM tags
        dve_ctr = [0]    # off-diag chunk counter for DVE assignment

        def exp_chunk(pt_dst, st_src, on_dve):
            """pt_dst (bf16 SBUF) = exp(SM_SCALE * st_src) (f32 PSUM)."""
            if "exp" in skip:
                nc.scalar.activation(
                    pt_dst[:, 0, 0:1], st_src[:, 0, 0:1], AF.Exp, scale=SM_SCALE
                )
                return
            if on_dve:
                nc.vector.tensor_scalar(
                    pt_dst.bitcast(I16), st_src, SCHRAU_A, SCHRAU_B,
                    ALU.mult, ALU.add,
                )
            else:
                nc.scalar.activation(pt_dst, st_src, AF.Exp, scale=SM_SCALE)

        rep_ctx = tc.For_i(0, repeat, 1) if repeat > 1 else None
        if rep_ctx is not None:
            ctx.enter_context(rep_ctx)

        def emit_pv_work(c):
            """Generate phase-B/epilogue work items for a finished block.

            Returns a list of zero-arg callables, in required program
            order; these get interleaved between the NEXT block's
            S/exp chunks so the PE never sits idle waiting on ACT.
            """
            p, qb, pt, vv_sb = c["p"], c["qb"], c["pt"], c["vv"]
            items = []

            def pv_group(s, qi):
                def f():
                    if c["u"] is None:
                        u = up.tile([NP, 4, 512], F32, tag="u", name="u")
                        c["u"] = u
                    qt_g = 4 * qb + qi
                    for ki in range(qt_g + 1):
                        nc.tensor.matmul(
                            c["u"][:, qi, 0:E + 1],
                            pt[:, s, ki, qi * NP:(qi + 1) * NP],
                            vv_sb[:, ki, :],
                            start=(ki == 0),
                            stop=(ki == qt_g),
                        )
                return f

            def u_copy(s):
                def f():
                    # one copy releases the PSUM banks; epilogue runs
                    # from SBUF far off the PE critical path
                    dst = ep.tile([NP, 4, E + 1], F32, tag=f"u{s}sb",
                                  name=f"u{s}sb")
                    c[f"u{s}sb"] = dst
                    nc.vector.tensor_copy(dst[:], c["u"][:, :, 0:E + 1])
                return f

            if "pv" not in skip:
                for s in range(2):
                    for qi in range(4):
                        items.append(pv_group(s, qi))
                    items.append(u_copy(s))
            items.append(lambda: emit_epilogue(c))
            return items

        def emit_epilogue(c):
            """DVE epilogue + finale + out-DMA for a completed block."""
            p, qb = c["p"], c["qb"]
            act_finale = (p == PAIRS - 1 and qb <= 1)
            o_qb = xp.tile([NP, 4, E], F32, tag="o")
            if "epi" in skip or "pv" in skip:
                nc.gpsimd.memset(o_qb[:], 0.0)
                nc.sync.dma_start(
                    out_d[p].rearrange("(t q) e -> q t e", q=NP)[
                        :, 4 * qb:4 * qb + 4, :
                    ],
                    o_qb[:],
                )
                return
            u0_sb, u1_sb = c["u0sb"], c["u1sb"]
            xb = xp.tile([NP, 4, E], F32, tag="xb")
            msb = ep.tile([NP, 4], F32, tag="msb")
            d1l = ep.tile([NP, 4], F32, tag="d1l")  # lam * den1
            nc.vector.tensor_scalar(
                d1l[:], u0_sb[:, :, 0], lam_sb[:, 0:1], None, ALU.mult
            )
            t2 = ep.tile([NP, 4, E], F32, tag="t2")  # lam*den1*U2
            nc.vector.tensor_tensor(
                t2[:],
                u1_sb[:, :, 1:E + 1],
                d1l[:].unsqueeze(2).broadcast_to([NP, 4, E]),
                ALU.mult,
            )
            xsq = ep.tile([NP, 4, E], F32, tag="xsq")
            for qi in range(4):
                # x = U1*den2 - t2
                nc.vector.scalar_tensor_tensor(
                    xb[:, qi, :],
                    u0_sb[:, qi, 1:E + 1],
                    u1_sb[:, qi, 0:1],
                    t2[:, qi, :],
                    ALU.mult,
                    ALU.subtract,
                )
                # ms = mean(x^2) via accum_out on the square pass
                nc.vector.scalar_tensor_tensor(
                    xsq[:, qi, :],
                    xb[:, qi, :],
                    1.0 / E,
                    xb[:, qi, :],
                    ALU.mult,
                    ALU.mult,
                    accum_out=msb[:, qi:qi + 1],
                )
            # eps correction: ms += eps*(den1*den2)^2
            dd = ep.tile([NP, 4], F32, tag="dd")
            nc.vector.scalar_tensor_tensor(
                dd[:],
                u0_sb[:, :, 0],
                float(np.sqrt(EPS)),
                u1_sb[:, :, 0],
                ALU.mult,
                ALU.mult,
            )
            edd = ep.tile([NP, 4], F32, tag="edd")
            nc.vector.tensor_tensor(edd[:], dd[:], dd[:], ALU.mult)
            ms = ep.tile([NP, 4], F32, tag="msq")
            nc.vector.tensor_tensor(ms[:], msb[:], edd[:], ALU.add)
            if act_finale:
                # tail blocks: ACT is idle there, and Ln/Exp share the
                # already-loaded activation table set.
                # rs = exp(-0.5*ln(ms) + ln(0.2)) = 0.2*rsqrt(ms)
                lg = ep.tile([NP, 4], F32, tag="lg")
                nc.scalar.activation(lg[:], ms[:], AF.Ln)
                rs = ep.tile([NP, 4], F32, tag="rs")
                nc.scalar.activation(rs[:], lg[:], AF.Exp, scale=-0.5,
                                     bias=float(np.log(OUT_SCALE)))
            else:
                # rs = 0.2*rsqrt(ms) via quake seed + 3 Newton steps
                # (DVE-only so the scalar engine stays exp-only)
                sh = ep.tile([NP, 4], I32, tag="sh")
                nc.vector.tensor_scalar(
                    sh[:], ms[:].bitcast(I32), 1, None,
                    ALU.logical_shift_right
                )
                y = ep.tile([NP, 4], F32, tag="y")
                nc.vector.tensor_tensor(
                    y[:].bitcast(I32),
                    magic[:].broadcast_to([NP, 4]).bitcast(I32),
                    sh[:],
                    ALU.subtract,
                )
                mh = ep.tile([NP, 4], F32, tag="mh")  # 0.5*ms
                nc.vector.tensor_scalar(mh[:], ms[:], 0.5, None, ALU.mult)
                for it in range(3):
                    yy = ep.tile([NP, 4], F32, tag="yy", name="yy")
                    nc.vector.tensor_tensor(yy[:], y[:], y[:], ALU.mult)
                    nc.vector.tensor_tensor(yy[:], yy[:], mh[:], ALU.mult)
                    fin = -OUT_SCALE if it == 2 else -1.0
                    nc.vector.tensor_scalar(
                        yy[:], yy[:], 1.5, fin, ALU.subtract, ALU.mult
                    )
                    yn = ep.tile([NP, 4], F32, tag="yn", name="yn")
                    nc.vector.tensor_tensor(yn[:], y[:], yy[:], ALU.mult)
                    y = yn
                rs = y
            nc.vector.tensor_tensor(
                o_qb[:],
                xb[:],
                rs[:].unsqueeze(2).broadcast_to([NP, 4, E]),
                ALU.mult,
            )
            if not w_is_ones:
                nc.vector.tensor_tensor(
                    o_qb[:],
                    o_qb[:],
                    wb_sb[:].unsqueeze(1).broadcast_to([NP, 4, E]),
                    ALU.mult,
                )
            nc.sync.dma_start(
                out_d[p].rearrange("(t q) e -> q t e", q=NP)[
                    :, 4 * qb:4 * qb + 4, :
                ],
                o_qb[:],
            )

        prev = None           # block awaiting PV/epilogue
        pending = []          # its work items
        emitted = [0]

        ios = []
        for p in range(PAIRS):
            qt_sb = io.tile([NP, L], BF16, tag=f"qt{p}", name="qt_sb")
            kt_sb = io.tile([NP, L], BF16, tag=f"kt{p}", name="kt_sb")
            vv_sb = io.tile([NP, NKT, E + 1], BF16, tag=f"vv{p}", name="vv_sb")
            # Split input DMAs so the first S matmul (qb=3, ki=0) can
            # start as soon as qt cols 1536: and kt cols 0:512 land.
            nc.sync.dma_start(kt_sb[:, 0:QB], kt_d[p, :, 0:QB])
            for qq in reversed(range(NQB)):
                nc.sync.dma_start(
                    qt_sb[:, qq * QB:(qq + 1) * QB],
                    qt_d[p, :, qq * QB:(qq + 1) * QB],
                )
            nc.sync.dma_start(kt_sb[:, QB:L], kt_d[p, :, QB:L])
            nc.sync.dma_start(vv_sb[:], vv_d[p].rearrange("(t k) e -> k t e", k=NP))
            ios.append((qt_sb, kt_sb, vv_sb))

        # Blocks in descending size, pairs interleaved: each A phase then
        # hosts the SAME-size PV of the other pair's previous block, so
        # the drizzled PE work matches the exp time it must cover; the
        # kernel tail is the smallest block's PV + epilogue. PV/epilogue
        # of the previous block are interleaved between S/exp chunks in
        # PROGRAM ORDER -- the PE queue is strict FIFO, so this is what
        # actually fills PE gaps while ACT works through the exps.
        for p in range(PAIRS):
            for qb in reversed(range(NQB)):
                qt_sb, kt_sb, vv_sb = ios[p]
                nki = 4 * qb + 4
                pt = ptp.tile([NP, 2, nki, QB], BF16, tag="pt", name="pt")
                cur = {"p": p, "qb": qb, "vv": vv_sb, "u": None, "pt": pt}
                for ki in range(nki):
                    c0 = max(0, ki - 4 * qb) * NP
                    st_par[0] = (st_par[0] + 1) % 2
                    st = stp.tile(
                        [NP, 2, QB], F32,
                        tag=f"st{st_par[0]}", name=f"st{st_par[0]}",
                    )
                    if "s" not in skip:
                        for s in range(2):
                            sp = slice(s * D, (s + 1) * D)
                            # tile_position (64*s, 0): the two streams
                            # run concurrently in the PE row-halves
                            nc.tensor.matmul(
                                st[:, s, c0:QB],
                                kt_sb[sp, ki * NP:(ki + 1) * NP],
                                qt_sb[sp, qb * QB + c0:(qb + 1) * QB],
                                start=True,
                                stop=True,
                            )
                    dve_ctr[0] += 1
                    on_dve = (dve_every > 0 and c0 == 0
                              and (dve_ctr[0] % dve_every == 0))
                    exp_chunk(pt[:, :, ki, c0:], st[:, :, c0:], on_dve)
                    if c0 > 0 and "mask" not in skip:
                        # diagonal subtile of this k-tile, both streams
                        sl = pt[:, :, ki, c0:c0 + NP]
                        nc.gpsimd.affine_select(
                            sl, sl,
                            pattern=[[0, 2], [1, NP]],
                            compare_op=ALU.is_ge,
                            fill=0.0,
                            base=0,
                            channel_multiplier=-1,
                        )
                    # drizzle in prev block's PV work (front-loaded so
                    # the epilogue chain starts before this A ends)
                    den = max(1, nki - 2)
                    target = min(len(pending),
                                 (len(pending) * (ki + 1) + den - 1) // den)
                    while emitted[0] < target:
                        pending[emitted[0]]()
                        emitted[0] += 1
                if "mask" not in skip:
                    # diagonal subtile of k-tile 4qb (qi=0), c0 was 0
                    sl = pt[:, :, 4 * qb, 0:NP]
                    nc.gpsimd.affine_select(
                        sl, sl,
                        pattern=[[0, 2], [1, NP]],
                        compare_op=ALU.is_ge,
                        fill=0.0,
                        base=0,
                        channel_multiplier=-1,
                    )
                prev = cur
                pending = emit_pv_work(cur)
                emitted[0] = 0

        # drain the last block
        while emitted[0] < len(pending):
            pending[emitted[0]]()
            emitted[0] += 1

    nc.compile()
    return nc


_PROGRAM_CACHE: dict = {}


def _get_program(w_is_ones: bool, repeat: int = 1, dve_every: int = 0) -> bass.Bass:
    key = (w_is_ones, repeat, dve_every)
    if key not in _PROGRAM_CACHE:
        _PROGRAM_CACHE[key] = _build_program(w_is_ones, repeat,
                                             dve_every=dve_every)
    return _PROGRAM_CACHE[key]


def make_in_maps(query, key, value, lambda_q1, lambda_k1, lambda_q2, lambda_k2,
                 sub_norm_w):
    """Host-side shard/pack. Returns (in_maps, w_is_ones)."""
    query = np.asarray(query, dtype=np.float32)
    key = np.asarray(key, dtype=np.float32)
    value = np.asarray(value, dtype=np.float32)
    lam = float(
        np.exp(np.sum(np.float64(lambda_q1) * np.float64(lambda_k1)))
        - np.exp(np.sum(np.float64(lambda_q2) * np.float64(lambda_k2)))
        + LAMBDA_INIT
    )
    w = np.asarray(sub_norm_w, dtype=np.float32)
    w_is_ones = bool(np.all(w == 1.0))

    import ml_dtypes

    bf16 = ml_dtypes.bfloat16
    q5 = query.reshape(B, L, H, 2 * D)
    k5 = key.reshape(B, L, H, 2 * D)
    v4 = value.reshape(B, L, H, E)
    lam_arr = np.full((NP, 1), lam, dtype=np.float32)
    wb = np.broadcast_to(w[None, :], (NP, E)).copy() if not w_is_ones else None

    in_maps = []
    for c in range(N_CORES):
        qt = np.empty((PAIRS, NP, L), dtype=bf16)
        kt = np.empty((PAIRS, NP, L), dtype=bf16)
        vv = np.empty((PAIRS, L, E + 1), dtype=bf16)
        for p in range(PAIRS):
            f = c * PAIRS + p
            b, h = divmod(f, H)
            qt[p] = q5[b, :, h].T.astype(bf16)
            kt[p] = k5[b, :, h].T.astype(bf16)
            vv[p, :, 0] = 1.0
            vv[p, :, 1:] = v4[b, :, h].astype(bf16)
        m = {"qt": qt, "kt": kt, "vv": vv, "lam": lam_arr}
        if not w_is_ones:
            m["wb"] = wb
        in_maps.append(m)
    return in_maps, w_is_ones


def assemble_output(results) -> np.ndarray:
    out = np.empty((B, L, H * E), dtype=np.float32)
    for c in range(N_CORES):
        o = results[c]["out"]
        for p in range(PAIRS):
            f = c * PAIRS + p
            b, h = divmod(f, H)
            out[b, :, h * E: (h + 1) * E] = o[p]
    return out


def kernel(query, key, value, lambda_q1, lambda_k1, lambda_q2, lambda_k2,
           sub_norm_w, **_unused):
    in_maps, w_is_ones = make_in_maps(
        query, key, value, lambda_q1, lambda_k1, lambda_q2, lambda_k2, sub_norm_w
    )
    nc = _get_program(w_is_ones)
    res = run_bass_kernel_spmd(nc, in_maps, core_ids=list(range(N_CORES)))
    return assemble_output(res.results)
